# revision 15
# baseline (speedup 1.0000x reference)
"""Fused multi-head attention + LayerNorm kernel for 8 Trainium2 NeuronCores.

Problem (hardcoded): B=4, S=2048, DIM=1024, H=16, HD=64; out = LayerNorm(
softmax(q W_q^T (k W_k^T)^T / sqrt(HD)) (v W_v^T) W_o^T + b_o) per reference.

Sharding: core c -> batch b = c//2, head-group g = c%2 (8 heads / 512 features).
The two cores of a pair exchange normalized attention outputs (AllGather over
pairs) so each finalizes half of the tokens.

Per-core dataflow (feature-major transposed layouts everywhere):
  1. q/k projections in bf16 -> qhT/khT [64(hd), 8(head), 2048(tok)];
     v projection in f32r -> vh_aug [128(j), 16(jt), 583] packed per head as
     64 values + a ones column (next head's data acts as harmless padding up
     to the 128-wide stationary).
  2. Attention per (head, j-tile): scoresT = khT^T qhT (PE, K=64 bf16),
     ET = exp(scale * scoresT) on ScalarE (psum -> sbuf f32r),
     outT_aug += vh_aug^T ET on PE (f32r); row 64 accumulates the softmax
     denominator thanks to the ones column.
  3. Rowsum reciprocal (DVE) -> gpsimd partition-broadcast -> normalize on
     eviction into out_normT (bf16).
  4. AllGather the cross token-half within the pair.
  5. Output projection (bf16) + bias + LayerNorm (bn_stats) -> y half.

Host tricks: each core's q tokens are permuted so "my half" is always columns
0:1024; Wo rows are rotated per core so the [mine, peer] chunk order matches.
"""
import sys

sys.path.insert(0, "/opt/trn_rl_repo")

import numpy as np
import ml_dtypes

B, S, DIM, H, HD = 4, 2048, 1024, 16, 64
NCORES = 8
NH = 8             # heads per core
FL = NH * HD       # 512 local features
EPS = 1e-5
SCALE = HD ** -0.5
P = 128
JT = S // P        # 16
IC = S // 512      # 4
TH = S // 2        # 1024 tokens finalized per core
DC = DIM // P      # 8 contraction chunks
HW = HD + 1        # 65: head block width in vh_aug
VW = NH * HW             # 520 (stationary windows spill into next block)

_cache = {}


def _build():
    import os
    STAGE = int(os.environ.get("STAGE", "4"))
    import concourse.bass as bass
    import concourse.bacc as bacc
    import concourse.tile as tile
    from concourse import mybir
    f32 = mybir.dt.float32
    f32r = mybir.dt.float32r
    bf16 = mybir.dt.bfloat16
    i32 = mybir.dt.int32
    EXPF = mybir.ActivationFunctionType.Exp
    SQRTF = mybir.ActivationFunctionType.Sqrt
    ALU = mybir.AluOpType
    ds = bass.ds

    nc = bacc.Bacc("TRN2", target_bir_lowering=False, debug=False,
                   num_devices=NCORES)

    xqT_d = nc.dram_tensor("xqT", [DIM, S], bf16, kind="ExternalInput")
    xkT_d = nc.dram_tensor("xkT", [DIM, S], bf16, kind="ExternalInput")
    xvT_d = nc.dram_tensor("xvT", [DIM, S], f32r, kind="ExternalInput")
    wqT_d = nc.dram_tensor("wqT", [DIM, FL], bf16, kind="ExternalInput")
    wkT_d = nc.dram_tensor("wkT", [DIM, FL], bf16, kind="ExternalInput")
    wvT_d = nc.dram_tensor("wvT", [DIM, FL], f32r, kind="ExternalInput")
    woT_d = nc.dram_tensor("woT", [DIM, DIM], bf16, kind="ExternalInput")
    bq_d = nc.dram_tensor("bq", [FL], f32, kind="ExternalInput")
    bk_d = nc.dram_tensor("bk", [FL], f32, kind="ExternalInput")
    bv_d = nc.dram_tensor("bv", [FL], f32, kind="ExternalInput")
    bo_d = nc.dram_tensor("bo", [DIM], f32, kind="ExternalInput")
    gamma_d = nc.dram_tensor("gamma", [DIM], f32, kind="ExternalInput")
    beta_d = nc.dram_tensor("beta", [DIM], f32, kind="ExternalInput")
    pidx_d = nc.dram_tensor("pidx", [1, 1], i32, kind="ExternalInput")
    y_d = nc.dram_tensor("y", [TH, DIM], f32, kind="ExternalOutput")

    PAIRS = [[0, 1], [2, 3], [4, 5], [6, 7]]

    def bcast_ap(ap, parts):
        return bass.AP(tensor=ap.tensor, offset=ap.offset,
                       ap=[[0, parts]] + list(ap.ap))

    with tile.TileContext(nc) as tc:
        import contextlib
        with contextlib.ExitStack() as ctx:
            persist = ctx.enter_context(tc.tile_pool(name="persist", bufs=1))
            ws = ctx.enter_context(tc.tile_pool(name="ws", bufs=1))
            xs = ctx.enter_context(tc.tile_pool(name="xs", bufs=4))
            et_pool = ctx.enter_context(tc.tile_pool(name="et", bufs=2))
            bc_pool = ctx.enter_context(tc.tile_pool(name="bc", bufs=1))
            ln_pool = ctx.enter_context(tc.tile_pool(name="ln", bufs=2))
            dram = ctx.enter_context(
                tc.tile_pool(name="dram", bufs=1, space="DRAM"))
            scp = ctx.enter_context(
                tc.tile_pool(name="scp", bufs=2, space="PSUM"))
            avp = ctx.enter_context(
                tc.tile_pool(name="avp", bufs=4, space="PSUM"))

            # ---------------- persistent state ----------------
            qhT = persist.tile([HD, NH, S], bf16)
            khT = persist.tile([HD, NH, S], bf16)
            vh_aug = persist.tile([P, JT * VW + (P - HW)], f32r)
            out_normT = persist.tile([P, FL // P, S], bf16)
            rs64 = persist.tile([HD + 1, 512], f32)   # row 64 stages sums

            # ones columns (rest of vh_aug holds data or harmless garbage;
            # garbage feeds only psum rows 65:127 which are never read)
            ones_f32 = persist.tile([P, P - HW], f32)
            nc.vector.memset(ones_f32, 1.0)
            vh_view = vh_aug[:, :JT * VW].rearrange("p (j w) -> p j w", w=VW)
            for h in range(NH):
                nc.scalar.copy(vh_view[:, :, h * HW + HD], ones_f32[:, :JT])
            # tail pad after the last head's window must be finite
            nc.scalar.copy(vh_aug[:, JT * VW:], ones_f32)

            bq_sb = persist.tile([P, FL // P], f32)
            bk_sb = persist.tile([P, FL // P], f32)
            nc.sync.dma_start(bq_sb, bq_d.rearrange("(a p) -> p a", p=P))
            nc.sync.dma_start(bk_sb, bk_d.rearrange("(a p) -> p a", p=P))
            bv_bc = persist.tile([P, FL], f32)
            nc.gpsimd.dma_start(bv_bc, bcast_ap(bv_d[:], P))
            bo_bc = persist.tile([P, DIM], f32)
            nc.gpsimd.dma_start(bo_bc, bcast_ap(bo_d[:], P))
            gamma_bc = persist.tile([P, DIM], f32)
            nc.gpsimd.dma_start(gamma_bc, bcast_ap(gamma_d[:], P))
            beta_bc = persist.tile([P, DIM], f32)
            nc.gpsimd.dma_start(beta_bc, bcast_ap(beta_d[:], P))
            eps_sb = persist.tile([P, 1], f32)
            nc.vector.memset(eps_sb, EPS)
            pidx_sb = persist.tile([1, 1], i32)
            nc.sync.dma_start(pidx_sb, pidx_d[:])

            # weights: wq slot reused by peer_T, wv slot reused by woT
            wq_sb = ws.tile([P, DC, FL], bf16, tag="wA")
            wk_sb = ws.tile([P, DC, FL], bf16, tag="wB")
            wv_sb = ws.tile([P, DC, FL], f32r, tag="wC")
            nc.sync.dma_start(wq_sb, wqT_d.rearrange("(a p) f -> p a f", p=P))
            nc.sync.dma_start(wk_sb, wkT_d.rearrange("(a p) f -> p a f", p=P))
            nc.sync.dma_start(wv_sb, wvT_d.rearrange("(a p) f -> p a f", p=P))

            # ---------------- q/k projections ----------------
            for name, x_d, w_sb, b_sb, dstT in (
                ("q", xqT_d, wq_sb, bq_sb, qhT),
                ("k", xkT_d, wk_sb, bk_sb, khT),
            ):
                for tci in range(IC):
                    xt = []
                    for dc in range(DC):
                        x_tile = xs.tile([P, 512], bf16, tag="xqk",
                                         name=f"x{name}_{tci}_{dc}")
                        nc.sync.dma_start(
                            x_tile,
                            x_d[dc * P:(dc + 1) * P, tci * 512:(tci + 1) * 512])
                        xt.append(x_tile)
                    for fc in range(FL // P):
                        ps = avp.tile([P, 512], f32, tag="ps512",
                                      name=f"ps_{name}_{tci}_{fc}")
                        for dc in range(DC):
                            nc.tensor.matmul(
                                ps, w_sb[:, dc, fc * P:(fc + 1) * P], xt[dc],
                                start=(dc == 0), stop=(dc == DC - 1))
                        for hf in range(2):
                            h = 2 * fc + hf
                            nc.vector.tensor_scalar_add(
                                dstT[:, h, tci * 512:(tci + 1) * 512],
                                ps[hf * HD:(hf + 1) * HD],
                                b_sb[hf * HD:(hf + 1) * HD, fc:fc + 1])

            # ---------------- v projection ----------------
            for jt in range(JT):
                ps = avp.tile([P, FL], f32, tag="ps512", name=f"ps_v_{jt}")
                for dc in range(DC):
                    xv_tile = xs.tile([P, P], f32r, tag="xv",
                                      name=f"xv_{jt}_{dc}")
                    nc.sync.dma_start(
                        xv_tile,
                        xvT_d[dc * P:(dc + 1) * P, jt * P:(jt + 1) * P])
                    nc.tensor.matmul(ps, xv_tile, wv_sb[:, dc, :],
                                     start=(dc == 0), stop=(dc == DC - 1))
                for h in range(NH):
                    nc.vector.scalar_tensor_tensor(
                        vh_aug[:, jt * VW + h * HW:jt * VW + h * HW + HD],
                        ps[:, h * HD:(h + 1) * HD], 0.0,
                        bv_bc[:, h * HD:(h + 1) * HD],
                        op0=ALU.add, op1=ALU.add)

            # ---------------- attention ----------------
            for h in range(NH if STAGE >= 2 else 0):
                pav = [avp.tile([P, 512], f32, tag="ps512",
                                name=f"pav_{h}_{i}") for i in range(IC)]
                for jt in range(JT):
                    for half in range(2):
                        psc = scp.tile([P, 1024], f32, tag="sc",
                                       name=f"sc_{h}_{jt}_{half}")
                        for i2 in range(2):
                            ic = half * 2 + i2
                            nc.tensor.matmul(
                                psc[:, i2 * 512:(i2 + 1) * 512],
                                khT[:, h, jt * P:(jt + 1) * P],
                                qhT[:, h, ic * 512:(ic + 1) * 512],
                                start=True, stop=True)
                        et = et_pool.tile([P, 1024], f32r, tag="et",
                                          name=f"et_{h}_{jt}_{half}")
                        nc.scalar.activation(et, psc, EXPF, scale=SCALE)
                        for i2 in range(2):
                            ic = half * 2 + i2
                            nc.tensor.matmul(
                                pav[ic],
                                vh_aug[:, jt * VW + h * HW:
                                       jt * VW + h * HW + P],
                                et[:, i2 * 512:(i2 + 1) * 512],
                                start=(jt == 0), stop=(jt == JT - 1))
                # evict raw (rows 0:64 data, row 64 = denominator)
                for ic in range(IC):
                    nc.vector.tensor_copy(
                        out_normT[(h % 2) * HD:(h % 2) * HD + HD,
                                  h // 2, ic * 512:(ic + 1) * 512],
                        pav[ic][:HD])
                # rowsums -> partition 0 -> reciprocal -> broadcast -> scale
                for ic in range(IC):
                    nc.vector.tensor_copy(rs64[HD:HD + 1], pav[ic][HD:HD + 1])
                    rh0 = bc_pool.tile([1, 2, 512], f32, tag="rh0",
                                       name=f"rh0_{h}_{ic}")
                    nc.sync.dma_start(rh0[:, 0, :], rs64[HD:HD + 1])
                    rrec = bc_pool.tile([1, 512], f32, tag="rrec",
                                        name=f"rrec_{h}_{ic}")
                    nc.vector.reciprocal_approx_accurate(
                        rrec, rh0[:, 0, :], rh0[:, 1, :])
                    rbc = bc_pool.tile([P, 512], f32, tag="rbc",
                                       name=f"rbc_{h}_{ic}")
                    nc.gpsimd.partition_broadcast(rbc, rrec)
                    hb = (h % 2) * HD
                    dst = out_normT[hb:hb + HD,
                                    h // 2, ic * 512:(ic + 1) * 512]
                    nc.vector.tensor_mul(dst, dst, rbc[hb:hb + HD])

            # ---------------- pair exchange ----------------
            cc_in = dram.tile([FL, TH], bf16)
            cc_out = dram.tile([2 * FL, TH], bf16)
            if STAGE >= 3:
                peer_T = ws.tile([P, FL // P, TH], bf16, tag="wA")
                for pi in range(FL // P):
                    nc.sync.dma_start(cc_in[pi * P:(pi + 1) * P, :],
                                      out_normT[:, pi, TH:S])
                nc.gpsimd.collective_compute(
                    "AllGather", ALU.bypass, replica_groups=PAIRS,
                    ins=[cc_in.opt()], outs=[cc_out.opt()])
                # peer = (slot0 + slot1) - mine, exact in f32, static reads
                for pi in range(FL // P):
                    ta = xs.tile([P, TH], bf16, tag="cca", bufs=2,
                                 name=f"cca_{pi}")
                    tb = xs.tile([P, TH], bf16, tag="ccb", bufs=2,
                                 name=f"ccb_{pi}")
                    nc.sync.dma_start(ta, cc_out[pi * P:(pi + 1) * P, :])
                    nc.sync.dma_start(tb, cc_out[FL + pi * P:
                                                 FL + (pi + 1) * P, :])
                    tmp = xs.tile([P, TH], f32, tag="cct", bufs=2,
                                  name=f"cct_{pi}")
                    nc.vector.tensor_add(tmp, ta, tb)
                    nc.vector.scalar_tensor_tensor(
                        peer_T[:, pi, :], tmp, 0.0, out_normT[:, pi, TH:S],
                        op0=ALU.add, op1=ALU.subtract)

            # ---------------- output projection + layernorm ----------------
            woT_sb = ws.tile([P, DC, DIM], bf16, tag="wC")
            nc.sync.dma_start(woT_sb, woT_d.rearrange("(a p) f -> p a f", p=P))
            for it in range(TH // P if STAGE >= 3 else 0):
                psf = [avp.tile([P, 512], f32, tag="ps512",
                                name=f"psf_{it}_{f}") for f in range(2)]
                for fcc in range(2):
                    for cc in range(DC):
                        if cc < 4:
                            stat = out_normT[:, cc, it * P:(it + 1) * P]
                        else:
                            stat = peer_T[:, cc - 4, it * P:(it + 1) * P]
                        nc.tensor.matmul(
                            psf[fcc], stat,
                            woT_sb[:, cc, fcc * 512:(fcc + 1) * 512],
                            start=(cc == 0), stop=(cc == DC - 1))
                xln = ln_pool.tile([P, DIM], f32, tag="xln", name=f"xln_{it}")
                for fcc in range(2):
                    nc.vector.scalar_tensor_tensor(
                        xln[:, fcc * 512:(fcc + 1) * 512], psf[fcc], 0.0,
                        bo_bc[:, fcc * 512:(fcc + 1) * 512],
                        op0=ALU.add, op1=ALU.add)
                stats = ln_pool.tile([P, 2, 6], f32, tag="st", name=f"st_{it}")
                for hf in range(2):
                    nc.vector.bn_stats(stats[:, hf, :],
                                       xln[:, hf * 512:(hf + 1) * 512])
                mv = ln_pool.tile([P, 2], f32, tag="mv", name=f"mv_{it}")
                nc.vector.bn_aggr(mv, stats)
                rstd = ln_pool.tile([P, 1], f32, tag="rstd", name=f"rstd_{it}")
                nc.scalar.activation(rstd, mv[:, 1:2], SQRTF, bias=eps_sb)
                nc.vector.reciprocal(rstd, rstd)
                nc.vector.scalar_tensor_tensor(
                    xln, xln, mv[:, 0:1], gamma_bc,
                    op0=ALU.subtract, op1=ALU.mult)
                nc.vector.scalar_tensor_tensor(
                    xln, xln, rstd, beta_bc, op0=ALU.mult, op1=ALU.add)
                nc.sync.dma_start(y_d[it * P:(it + 1) * P, :], xln)

    nc.compile()
    return nc


def kernel(q, k, v, Wq, bq, Wk, bk, Wv, bv, Wo, bo, gamma, beta):
    from concourse.bass_utils import run_bass_kernel_spmd

    if "nc" not in _cache:
        _cache["nc"] = _build()
    nc = _cache["nc"]

    q = np.asarray(q, np.float32)
    k = np.asarray(k, np.float32)
    v = np.asarray(v, np.float32)
    Wq = np.asarray(Wq, np.float32); Wk = np.asarray(Wk, np.float32)
    Wv = np.asarray(Wv, np.float32); Wo = np.asarray(Wo, np.float32)
    bf = ml_dtypes.bfloat16
    WoT = Wo.T  # [c, f]
    in_maps = []
    for c in range(NCORES):
        b, g = c // 2, c % 2
        fsl = slice(g * FL, (g + 1) * FL)
        # my token half first (so "my half" is always columns 0:TH)
        xq = q[b].T if g == 0 else np.concatenate(
            [q[b].T[:, TH:], q[b].T[:, :TH]], axis=1)
        # Wo rows rotated: [my features, peer features]
        woT = np.concatenate([WoT[g * FL:(g + 1) * FL],
                              WoT[(1 - g) * FL:(2 - g) * FL]], axis=0)
        in_maps.append({
            "xqT": np.ascontiguousarray(xq).astype(bf),
            "xkT": np.ascontiguousarray(k[b].T).astype(bf),
            "xvT": np.ascontiguousarray(v[b].T).astype(np.float32),
            "wqT": np.ascontiguousarray(Wq[fsl, :].T).astype(bf),
            "wkT": np.ascontiguousarray(Wk[fsl, :].T).astype(bf),
            "wvT": np.ascontiguousarray(Wv[fsl, :].T).astype(np.float32),
            "woT": np.ascontiguousarray(woT).astype(bf),
            "bq": np.asarray(bq, np.float32)[fsl],
            "bk": np.asarray(bk, np.float32)[fsl],
            "bv": np.asarray(bv, np.float32)[fsl],
            "bo": np.asarray(bo, np.float32),
            "gamma": np.asarray(gamma, np.float32),
            "beta": np.asarray(beta, np.float32),
            "pidx": np.array([[1 - g]], np.int32),
        })
    res = run_bass_kernel_spmd(nc, in_maps, list(range(NCORES)),
                               trace=_cache.get("trace", False))
    _cache["last_res"] = res
    y = np.empty((B, S, DIM), np.float32)
    for c in range(NCORES):
        b, g = c // 2, c % 2
        y[b, g * TH:(g + 1) * TH, :] = res.results[c]["y"]
    return y


# revision 17
# speedup vs baseline: 1.0099x; 1.0099x over previous
"""Fused multi-head attention + LayerNorm kernel for 8 Trainium2 NeuronCores.

Problem (hardcoded): B=4, S=2048, DIM=1024, H=16, HD=64; out = LayerNorm(
softmax(q W_q^T (k W_k^T)^T / sqrt(HD)) (v W_v^T) W_o^T + b_o) per reference.

Sharding: core c -> batch b = c//2, head-group g = c%2 (8 heads / 512 features).
The two cores of a pair exchange normalized attention outputs (AllGather over
pairs) so each finalizes half of the tokens.

Per-core dataflow (feature-major transposed layouts everywhere):
  1. q/k projections in bf16 -> qhT/khT [64(hd), 8(head), 2048(tok)];
     v projection in f32r -> vh_aug [128(j), 16(jt), 583] packed per head as
     64 values + a ones column (next head's data acts as harmless padding up
     to the 128-wide stationary).
  2. Attention per (head, j-tile): scoresT = khT^T qhT (PE, K=64 bf16),
     ET = exp(scale * scoresT) on ScalarE (psum -> sbuf f32r),
     outT_aug += vh_aug^T ET on PE (f32r); row 64 accumulates the softmax
     denominator thanks to the ones column.
  3. Rowsum reciprocal (DVE) -> gpsimd partition-broadcast -> normalize on
     eviction into out_normT (bf16).
  4. AllGather the cross token-half within the pair.
  5. Output projection (bf16) + bias + LayerNorm (bn_stats) -> y half.

Host tricks: each core's q tokens are permuted so "my half" is always columns
0:1024; Wo rows are rotated per core so the [mine, peer] chunk order matches.
"""
import sys

sys.path.insert(0, "/opt/trn_rl_repo")

import numpy as np
import ml_dtypes

B, S, DIM, H, HD = 4, 2048, 1024, 16, 64
NCORES = 8
NH = 8             # heads per core
FL = NH * HD       # 512 local features
EPS = 1e-5
SCALE = HD ** -0.5
P = 128
JT = S // P        # 16
IC = S // 512      # 4
TH = S // 2        # 1024 tokens finalized per core
DC = DIM // P      # 8 contraction chunks
HW = HD + 1        # 65: head block width in vh_aug
VW = NH * HW             # 520 (stationary windows spill into next block)

_cache = {}


def _build():
    import os
    STAGE = int(os.environ.get("STAGE", "4"))
    import concourse.bass as bass
    import concourse.bacc as bacc
    import concourse.tile as tile
    from concourse import mybir
    f32 = mybir.dt.float32
    f32r = mybir.dt.float32r
    bf16 = mybir.dt.bfloat16
    i32 = mybir.dt.int32
    EXPF = mybir.ActivationFunctionType.Exp
    SQRTF = mybir.ActivationFunctionType.Sqrt
    ALU = mybir.AluOpType
    ds = bass.ds

    nc = bacc.Bacc("TRN2", target_bir_lowering=False, debug=False,
                   num_devices=NCORES)

    xqT_d = nc.dram_tensor("xqT", [DIM, S], bf16, kind="ExternalInput")
    xkT_d = nc.dram_tensor("xkT", [DIM, S], bf16, kind="ExternalInput")
    xvT_d = nc.dram_tensor("xvT", [DIM, S], f32r, kind="ExternalInput")
    wqT_d = nc.dram_tensor("wqT", [DIM, FL], bf16, kind="ExternalInput")
    wkT_d = nc.dram_tensor("wkT", [DIM, FL], bf16, kind="ExternalInput")
    wvT_d = nc.dram_tensor("wvT", [DIM, FL], f32r, kind="ExternalInput")
    woT_d = nc.dram_tensor("woT", [DIM, DIM], bf16, kind="ExternalInput")
    bq_d = nc.dram_tensor("bq", [FL], f32, kind="ExternalInput")
    bk_d = nc.dram_tensor("bk", [FL], f32, kind="ExternalInput")
    bv_d = nc.dram_tensor("bv", [FL], f32, kind="ExternalInput")
    bo_d = nc.dram_tensor("bo", [DIM], f32, kind="ExternalInput")
    gamma_d = nc.dram_tensor("gamma", [DIM], f32, kind="ExternalInput")
    beta_d = nc.dram_tensor("beta", [DIM], f32, kind="ExternalInput")
    pidx_d = nc.dram_tensor("pidx", [1, 1], i32, kind="ExternalInput")
    y_d = nc.dram_tensor("y", [TH, DIM], f32, kind="ExternalOutput")

    PAIRS = [[0, 1], [2, 3], [4, 5], [6, 7]]

    def bcast_ap(ap, parts):
        return bass.AP(tensor=ap.tensor, offset=ap.offset,
                       ap=[[0, parts]] + list(ap.ap))

    with tile.TileContext(nc) as tc:
        import contextlib
        with contextlib.ExitStack() as ctx:
            persist = ctx.enter_context(tc.tile_pool(name="persist", bufs=1))
            ws = ctx.enter_context(tc.tile_pool(name="ws", bufs=1))
            xs = ctx.enter_context(tc.tile_pool(name="xs", bufs=4))
            et_pool = ctx.enter_context(tc.tile_pool(name="et", bufs=2))
            bc_pool = ctx.enter_context(tc.tile_pool(name="bc", bufs=1))
            ln_pool = ctx.enter_context(tc.tile_pool(name="ln", bufs=2))
            dram = ctx.enter_context(
                tc.tile_pool(name="dram", bufs=1, space="DRAM"))
            scp = ctx.enter_context(
                tc.tile_pool(name="scp", bufs=2, space="PSUM"))
            avp = ctx.enter_context(
                tc.tile_pool(name="avp", bufs=4, space="PSUM"))

            # ---------------- persistent state ----------------
            qhT = persist.tile([HD, NH, S], bf16)
            khT = persist.tile([HD, NH, S], bf16)
            vh_aug = persist.tile([P, JT * VW + (P - HW)], f32r)
            out_normT = persist.tile([P, FL // P, S], bf16)
            rs64 = persist.tile([HD + 1, 512], f32)   # row 64 stages sums

            # ones columns (rest of vh_aug holds data or harmless garbage;
            # garbage feeds only psum rows 65:127 which are never read)
            ones_f32 = persist.tile([P, P - HW], f32)
            nc.vector.memset(ones_f32, 1.0)
            vh_view = vh_aug[:, :JT * VW].rearrange("p (j w) -> p j w", w=VW)
            for h in range(NH):
                nc.scalar.copy(vh_view[:, :, h * HW + HD], ones_f32[:, :JT])
            # tail pad after the last head's window must be finite
            nc.scalar.copy(vh_aug[:, JT * VW:], ones_f32)

            bq_sb = persist.tile([P, FL // P], f32)
            bk_sb = persist.tile([P, FL // P], f32)
            nc.sync.dma_start(bq_sb, bq_d.rearrange("(a p) -> p a", p=P))
            nc.sync.dma_start(bk_sb, bk_d.rearrange("(a p) -> p a", p=P))
            bv_bc = persist.tile([P, FL], f32)
            nc.gpsimd.dma_start(bv_bc, bcast_ap(bv_d[:], P))
            bo_bc = persist.tile([P, DIM], f32)
            nc.gpsimd.dma_start(bo_bc, bcast_ap(bo_d[:], P))
            gamma_bc = persist.tile([P, DIM], f32)
            nc.gpsimd.dma_start(gamma_bc, bcast_ap(gamma_d[:], P))
            beta_bc = persist.tile([P, DIM], f32)
            nc.gpsimd.dma_start(beta_bc, bcast_ap(beta_d[:], P))
            eps_sb = persist.tile([P, 1], f32)
            nc.vector.memset(eps_sb, EPS)
            pidx_sb = persist.tile([1, 1], i32)
            nc.sync.dma_start(pidx_sb, pidx_d[:])

            # weights: wq slot reused by peer_T, wv slot reused by woT
            wq_sb = ws.tile([P, DC, FL], bf16, tag="wA")
            wk_sb = ws.tile([P, DC, FL], bf16, tag="wB")
            wv_sb = ws.tile([P, DC, FL], f32r, tag="wC")
            nc.sync.dma_start(wq_sb, wqT_d.rearrange("(a p) f -> p a f", p=P))
            nc.sync.dma_start(wk_sb, wkT_d.rearrange("(a p) f -> p a f", p=P))
            nc.sync.dma_start(wv_sb, wvT_d.rearrange("(a p) f -> p a f", p=P))

            # ---------------- q/k projections ----------------
            scope_stack = []
            def enter_scope(nm):
                while scope_stack:
                    n0, sid = scope_stack.pop()
                    nc.leave_named_scope(n0, sid, False)
                scope_stack.append((nm, nc.enter_named_scope(nm, False)[0]))
            enter_scope("proj_qk")
            for name, x_d, w_sb, b_sb, dstT in (
                ("q", xqT_d, wq_sb, bq_sb, qhT),
                ("k", xkT_d, wk_sb, bk_sb, khT),
            ):
                for tci in range(IC):
                    xt = []
                    for dc in range(DC):
                        x_tile = xs.tile([P, 512], bf16, tag="xqk",
                                         name=f"x{name}_{tci}_{dc}")
                        nc.sync.dma_start(
                            x_tile,
                            x_d[dc * P:(dc + 1) * P, tci * 512:(tci + 1) * 512])
                        xt.append(x_tile)
                    for fc in range(FL // P):
                        ps = avp.tile([P, 512], f32, tag="ps512",
                                      name=f"ps_{name}_{tci}_{fc}")
                        for dc in range(DC):
                            nc.tensor.matmul(
                                ps, w_sb[:, dc, fc * P:(fc + 1) * P], xt[dc],
                                start=(dc == 0), stop=(dc == DC - 1))
                        for hf in range(2):
                            h = 2 * fc + hf
                            nc.vector.tensor_scalar_add(
                                dstT[:, h, tci * 512:(tci + 1) * 512],
                                ps[hf * HD:(hf + 1) * HD],
                                b_sb[hf * HD:(hf + 1) * HD, fc:fc + 1])

            # ---------------- v projection ----------------
            enter_scope("proj_v")
            for jt in range(JT):
                ps = avp.tile([P, FL], f32, tag="ps512", name=f"ps_v_{jt}")
                for dc in range(DC):
                    xv_tile = xs.tile([P, P], f32r, tag="xv",
                                      name=f"xv_{jt}_{dc}")
                    nc.sync.dma_start(
                        xv_tile,
                        xvT_d[dc * P:(dc + 1) * P, jt * P:(jt + 1) * P])
                    nc.tensor.matmul(ps, xv_tile, wv_sb[:, dc, :],
                                     start=(dc == 0), stop=(dc == DC - 1))
                for h in range(NH):
                    nc.vector.scalar_tensor_tensor(
                        vh_aug[:, jt * VW + h * HW:jt * VW + h * HW + HD],
                        ps[:, h * HD:(h + 1) * HD], 0.0,
                        bv_bc[:, h * HD:(h + 1) * HD],
                        op0=ALU.add, op1=ALU.add)

            # ---------------- attention ----------------
            enter_scope("attn")
            for h in range(NH if STAGE >= 2 else 0):
                pav = [avp.tile([P, 512], f32, tag="ps512",
                                name=f"pav_{h}_{i}") for i in range(IC)]
                for jt in range(JT):
                    for half in range(2):
                        psc = scp.tile([P, 1024], f32, tag="sc",
                                       name=f"sc_{h}_{jt}_{half}")
                        for i2 in range(2):
                            ic = half * 2 + i2
                            nc.tensor.matmul(
                                psc[:, i2 * 512:(i2 + 1) * 512],
                                khT[:, h, jt * P:(jt + 1) * P],
                                qhT[:, h, ic * 512:(ic + 1) * 512],
                                start=True, stop=True)
                        et = et_pool.tile([P, 1024], f32r, tag="et",
                                          name=f"et_{h}_{jt}_{half}")
                        nc.scalar.activation(et, psc, EXPF, scale=SCALE)
                        for i2 in range(2):
                            ic = half * 2 + i2
                            nc.tensor.matmul(
                                pav[ic],
                                vh_aug[:, jt * VW + h * HW:
                                       jt * VW + h * HW + P],
                                et[:, i2 * 512:(i2 + 1) * 512],
                                start=(jt == 0), stop=(jt == JT - 1))
                # evict raw (rows 0:64 data, row 64 = denominator)
                for ic in range(IC):
                    nc.vector.tensor_copy(
                        out_normT[(h % 2) * HD:(h % 2) * HD + HD,
                                  h // 2, ic * 512:(ic + 1) * 512],
                        pav[ic][:HD])
                # rowsums -> partition 0 -> reciprocal -> broadcast -> scale
                for ic in range(IC):
                    nc.vector.tensor_copy(rs64[HD:HD + 1], pav[ic][HD:HD + 1])
                    rh0 = bc_pool.tile([1, 2, 512], f32, tag="rh0",
                                       name=f"rh0_{h}_{ic}")
                    nc.sync.dma_start(rh0[:, 0, :], rs64[HD:HD + 1])
                    rrec = bc_pool.tile([1, 512], f32, tag="rrec",
                                        name=f"rrec_{h}_{ic}")
                    nc.vector.reciprocal_approx_accurate(
                        rrec, rh0[:, 0, :], rh0[:, 1, :])
                    rbc = bc_pool.tile([P, 512], f32, tag="rbc",
                                       name=f"rbc_{h}_{ic}")
                    nc.gpsimd.partition_broadcast(rbc, rrec)
                    hb = (h % 2) * HD
                    dst = out_normT[hb:hb + HD,
                                    h // 2, ic * 512:(ic + 1) * 512]
                    nc.vector.tensor_mul(dst, dst, rbc[hb:hb + HD])

            # ---------------- pair exchange ----------------
            enter_scope("exch")
            cc_in = dram.tile([FL, TH], bf16)
            cc_out = dram.tile([2 * FL, TH], bf16)
            if STAGE >= 3:
                peer_T = ws.tile([P, FL // P, TH], bf16, tag="wA")
                for pi in range(FL // P):
                    nc.sync.dma_start(cc_in[pi * P:(pi + 1) * P, :],
                                      out_normT[:, pi, TH:S])
                nc.gpsimd.collective_compute(
                    "AllGather", ALU.bypass, replica_groups=PAIRS,
                    ins=[cc_in.opt()], outs=[cc_out.opt()])
                # peer = (slot0 + slot1) - mine, exact in f32, static reads
                for pi in range(FL // P):
                    ta = xs.tile([P, TH], bf16, tag="cca", bufs=2,
                                 name=f"cca_{pi}")
                    tb = xs.tile([P, TH], bf16, tag="ccb", bufs=2,
                                 name=f"ccb_{pi}")
                    nc.sync.dma_start(ta, cc_out[pi * P:(pi + 1) * P, :])
                    nc.sync.dma_start(tb, cc_out[FL + pi * P:
                                                 FL + (pi + 1) * P, :])
                    tmp = xs.tile([P, TH], f32, tag="cct", bufs=2,
                                  name=f"cct_{pi}")
                    nc.vector.tensor_add(tmp, ta, tb)
                    nc.vector.scalar_tensor_tensor(
                        peer_T[:, pi, :], tmp, 0.0, out_normT[:, pi, TH:S],
                        op0=ALU.add, op1=ALU.subtract)

            # ---------------- output projection + layernorm ----------------
            enter_scope("final")
            woT_sb = ws.tile([P, DC, DIM], bf16, tag="wC")
            nc.sync.dma_start(woT_sb, woT_d.rearrange("(a p) f -> p a f", p=P))
            for it in range(TH // P if STAGE >= 3 else 0):
                psf = [avp.tile([P, 512], f32, tag="ps512",
                                name=f"psf_{it}_{f}") for f in range(2)]
                for fcc in range(2):
                    for cc in range(DC):
                        if cc < 4:
                            stat = out_normT[:, cc, it * P:(it + 1) * P]
                        else:
                            stat = peer_T[:, cc - 4, it * P:(it + 1) * P]
                        nc.tensor.matmul(
                            psf[fcc], stat,
                            woT_sb[:, cc, fcc * 512:(fcc + 1) * 512],
                            start=(cc == 0), stop=(cc == DC - 1))
                xln = ln_pool.tile([P, DIM], f32, tag="xln", name=f"xln_{it}")
                for fcc in range(2):
                    nc.vector.scalar_tensor_tensor(
                        xln[:, fcc * 512:(fcc + 1) * 512], psf[fcc], 0.0,
                        bo_bc[:, fcc * 512:(fcc + 1) * 512],
                        op0=ALU.add, op1=ALU.add)
                stats = ln_pool.tile([P, 2, 6], f32, tag="st", name=f"st_{it}")
                for hf in range(2):
                    nc.vector.bn_stats(stats[:, hf, :],
                                       xln[:, hf * 512:(hf + 1) * 512])
                mv = ln_pool.tile([P, 2], f32, tag="mv", name=f"mv_{it}")
                nc.vector.bn_aggr(mv, stats)
                rstd = ln_pool.tile([P, 1], f32, tag="rstd", name=f"rstd_{it}")
                nc.scalar.activation(rstd, mv[:, 1:2], SQRTF, bias=eps_sb)
                nc.vector.reciprocal(rstd, rstd)
                nc.vector.scalar_tensor_tensor(
                    xln, xln, mv[:, 0:1], gamma_bc,
                    op0=ALU.subtract, op1=ALU.mult)
                nc.vector.scalar_tensor_tensor(
                    xln, xln, rstd, beta_bc, op0=ALU.mult, op1=ALU.add)
                nc.sync.dma_start(y_d[it * P:(it + 1) * P, :], xln)
            while scope_stack:
                n0, sid = scope_stack.pop()
                nc.leave_named_scope(n0, sid, False)

    nc.compile()
    return nc


def kernel(q, k, v, Wq, bq, Wk, bk, Wv, bv, Wo, bo, gamma, beta):
    from concourse.bass_utils import run_bass_kernel_spmd

    if "nc" not in _cache:
        _cache["nc"] = _build()
    nc = _cache["nc"]

    q = np.asarray(q, np.float32)
    k = np.asarray(k, np.float32)
    v = np.asarray(v, np.float32)
    Wq = np.asarray(Wq, np.float32); Wk = np.asarray(Wk, np.float32)
    Wv = np.asarray(Wv, np.float32); Wo = np.asarray(Wo, np.float32)
    bf = ml_dtypes.bfloat16
    WoT = Wo.T  # [c, f]
    in_maps = []
    for c in range(NCORES):
        b, g = c // 2, c % 2
        fsl = slice(g * FL, (g + 1) * FL)
        # my token half first (so "my half" is always columns 0:TH)
        xq = q[b].T if g == 0 else np.concatenate(
            [q[b].T[:, TH:], q[b].T[:, :TH]], axis=1)
        # Wo rows rotated: [my features, peer features]
        woT = np.concatenate([WoT[g * FL:(g + 1) * FL],
                              WoT[(1 - g) * FL:(2 - g) * FL]], axis=0)
        in_maps.append({
            "xqT": np.ascontiguousarray(xq).astype(bf),
            "xkT": np.ascontiguousarray(k[b].T).astype(bf),
            "xvT": np.ascontiguousarray(v[b].T).astype(np.float32),
            "wqT": np.ascontiguousarray(Wq[fsl, :].T).astype(bf),
            "wkT": np.ascontiguousarray(Wk[fsl, :].T).astype(bf),
            "wvT": np.ascontiguousarray(Wv[fsl, :].T).astype(np.float32),
            "woT": np.ascontiguousarray(woT).astype(bf),
            "bq": np.asarray(bq, np.float32)[fsl],
            "bk": np.asarray(bk, np.float32)[fsl],
            "bv": np.asarray(bv, np.float32)[fsl],
            "bo": np.asarray(bo, np.float32),
            "gamma": np.asarray(gamma, np.float32),
            "beta": np.asarray(beta, np.float32),
            "pidx": np.array([[1 - g]], np.int32),
        })
    res = run_bass_kernel_spmd(nc, in_maps, list(range(NCORES)),
                               trace=_cache.get("trace", False))
    _cache["last_res"] = res
    y = np.empty((B, S, DIM), np.float32)
    for c in range(NCORES):
        b, g = c // 2, c % 2
        y[b, g * TH:(g + 1) * TH, :] = res.results[c]["y"]
    return y
